# revision 8
# baseline (speedup 1.0000x reference)
"""DigitCaps dynamic-routing kernel for 8 Trainium2 NeuronCores.

Problem: x(32,16384,8) f32, W(10,16384,8,16) f32 -> v(32,10,16) f32
  u_hat[b,j,p,o] = sum_d x[b,p,d] W[j,p,d,o]   (never materialized!)
  3 routing iterations (softmax over j, weighted sums over p).

Strategy: shard P=16384 over 8 cores (P_loc=2048). Per routing iteration:
  s_part[b,j,o] = sum_{p,d} (c*x)[b,j,p,d] * W[j,p,d,o]     (PE, K=p 128-chunks)
  AllReduce s (20KB) -> v = squash(s)
  z[b,j,p,d]  = sum_o W[j,p,d,o] v[b,j,o]                   (PE, K=(d,o)=128 with
                                                             block-diagonal v rhs)
  uv[b,j,p]   = sum_d x[b,p,d] z[...]                        (DVE)
  bb += uv ; c = softmax_j(bb)
Iteration 1 uses c = 0.1 exactly. Final squash + cross-core s-sum on host.

Per-core SBUF layouts (p^ = p % 128 on partitions, t = p//128 in 0..15):
  xt  [128, t16, d8, b32]        ws [128, t16, d8, j10, o16]
  wz  [j10, 128=(d*16+o), t16, p128]   (DRAM, streamed per j)
"""
import numpy as np
from functools import lru_cache

import concourse.bacc as bacc
import concourse.mybir as mybir
from concourse import tile
from concourse.bass_utils import run_bass_kernel_spmd

F32 = mybir.dt.float32
AX = mybir.AxisListType
ALU = mybir.AluOpType
ACTF = mybir.ActivationFunctionType

B, J, P, D, O = 32, 10, 16384, 8, 16
NCORES = 8
PL = P // NCORES          # 2048
T = PL // 128             # 16 tiles of 128 p's
TG = 4                    # t-group size in z-phase
JO = J * O                # 160


def _emit(nc, n_cores):
    xt = nc.dram_tensor("xt", [128, T, D, B], F32, kind="ExternalInput")
    ws = nc.dram_tensor("ws", [128, T, D, J, O], F32, kind="ExternalInput")
    wz = nc.dram_tensor("wz", [J, 128, T, 128], F32, kind="ExternalInput")
    s3p = nc.dram_tensor("s3p", [B, JO], F32, kind="ExternalOutput")

    with tile.TileContext(nc) as tc:
        with (
            tc.tile_pool(name="per", bufs=1) as per,        # persistent
            tc.tile_pool(name="wsst", bufs=3) as wsst,      # ws stream
            tc.tile_pool(name="wzst", bufs=3) as wzst,      # wz stream
            tc.tile_pool(name="yp", bufs=2) as yp,
            tc.tile_pool(name="zc", bufs=2) as zc,          # z consume bufs
            tc.tile_pool(name="small", bufs=2) as small,
            tc.tile_pool(name="sps", bufs=2, space="PSUM") as sps,
            tc.tile_pool(name="zps", bufs=2, space="PSUM") as zps,
            tc.tile_pool(name="dram", bufs=2, space="DRAM") as dramp,
        ):
            x_sb = per.tile([128, T, D, B], F32)
            nc.sync.dma_start(x_sb[:], xt[:, :, :, :])
            # block-diagonal v holder: rows (d*16+o), cols per j (d*32+b).
            # Off-diagonal stays zero forever; blocks are re-DMA'd per iter.
            vblk = per.tile([128, J, D * B], F32)
            nc.gpsimd.memset(vblk[:], 0.0)
            bb = per.tile([128, T, J, B], F32)      # routing logits
            e_sb = per.tile([128, T, J, B], F32)    # exp(bb)
            c_sb = per.tile([128, T, J, B], F32)    # softmax coeffs
            se = per.tile([128, T, B], F32)         # sum_j exp
            rec = per.tile([128, T, B], F32)        # 1/sum

            for it in range(3):
                # ---------------- s-phase ----------------
                s_ps = sps.tile([B, JO], F32)
                if it > 0:
                    # softmax over j: c = exp(bb) / sum_j exp(bb)
                    nc.scalar.activation(e_sb[:], bb[:], ACTF.Exp)
                    nc.vector.tensor_reduce(
                        se[:, :, :, None],
                        e_sb.rearrange("p t j b -> p t b j"),
                        AX.X, ALU.add,
                    )
                    nc.vector.reciprocal(rec[:], se[:])
                    nc.vector.tensor_mul(
                        c_sb[:], e_sb[:],
                        rec[:, :, None, :].broadcast_to([128, T, J, B]),
                    )
                for t in range(T):
                    wst = wsst.tile([128, D, J, O], F32)
                    nc.sync.dma_start(wst[:], ws[:, t, :, :, :])
                    if it == 0:
                        # c == 0.1 exactly: lhsT = x, scale folded into copy
                        for d in range(D):
                            nc.tensor.matmul(
                                s_ps[:, :],
                                x_sb[:, t, d, :],
                                wst[:, d, :, :].rearrange("p j o -> p (j o)"),
                                start=(t == 0 and d == 0),
                                stop=(t == T - 1 and d == D - 1),
                            )
                    else:
                        y_t = yp.tile([128, J, D, B], F32)
                        nc.vector.tensor_mul(
                            y_t[:],
                            c_sb[:, t, :, None, :].broadcast_to([128, J, D, B]),
                            x_sb[:, t, None, :, :].broadcast_to([128, J, D, B]),
                        )
                        for j in range(J):
                            for d in range(D):
                                # one accumulation group for the whole bank:
                                # start marks the 2KB zero-region pending-zero,
                                # each slice's first write then overwrites
                                nc.tensor.matmul(
                                    s_ps[:, j * O:(j + 1) * O],
                                    y_t[:, j, d, :],
                                    wst[:, d, j, :],
                                    start=(t == 0 and j == 0 and d == 0),
                                    stop=(t == T - 1 and j == J - 1 and d == D - 1),
                                )
                s_sb = small.tile([B, JO], F32)
                nc.scalar.activation(s_sb[:], s_ps[:], ACTF.Copy,
                                     scale=0.1 if it == 0 else 1.0)
                if it == 2:
                    nc.sync.dma_start(s3p[:, :], s_sb[:])
                    break

                # ---------------- AllReduce s ----------------
                cc_in = dramp.tile([B, JO], F32)
                cc_out = dramp.tile([B, JO], F32)
                nc.sync.dma_start(cc_in[:], s_sb[:])
                nc.gpsimd.collective_compute(
                    "AllReduce", ALU.add,
                    replica_groups=[list(range(n_cores))],
                    ins=[cc_in[:].opt()], outs=[cc_out[:].opt()],
                )
                s_f = small.tile([B, JO], F32)
                nc.sync.dma_start(s_f[:], cc_out[:])

                # ---------------- squash -> v ----------------
                t2 = small.tile([B, JO], F32)
                nc.vector.tensor_mul(t2[:], s_f[:], s_f[:])
                sq = small.tile([B, J], F32)
                nc.vector.tensor_reduce(
                    sq[:, :, None], t2.rearrange("b (j o) -> b j o", j=J),
                    AX.X, ALU.add)
                r_ = small.tile([B, J], F32)
                nc.scalar.activation(r_[:], sq[:], ACTF.Sqrt)
                den = small.tile([B, J], F32)
                # den = (sq + 1) * r
                nc.vector.scalar_tensor_tensor(
                    den[:], sq[:], 1.0, r_[:], ALU.add, ALU.mult)
                rc2 = small.tile([B, J], F32)
                nc.vector.reciprocal(rc2[:], den[:])
                f_ = small.tile([B, J], F32)
                nc.vector.tensor_mul(f_[:], sq[:], rc2[:])
                v_sb = small.tile([B, J, O], F32)
                nc.vector.tensor_mul(
                    v_sb[:], s_f.rearrange("b (j o) -> b j o", j=J),
                    f_[:, :, None].broadcast_to([B, J, O]))
                # bounce v through DRAM, then scatter transposed copies into
                # the block-diagonal slots (DMA is exempt from the 32-aligned
                # partition-start rule that engine ops have)
                v_dr = dramp.tile([B, J, O], F32)
                nc.sync.dma_start(v_dr[:], v_sb[:])
                for j in range(J):
                    for d in range(D):
                        nc.sync.dma_start(
                            vblk[d * O:(d + 1) * O, j, d * B:(d + 1) * B],
                            v_dr[:, j, :].rearrange("b o -> o b"))

                # ---------------- z / uv phase ----------------
                for j in range(J):
                    wzs = wzst.tile([128, T, 128], F32)
                    nc.sync.dma_start(wzs[:], wz[j, :, :, :])
                    for tg in range(T // TG):
                        z_ps = zps.tile([128, TG, D * B], F32)
                        for t4 in range(TG):
                            # two 1KB outputs share each 2KB psum bank -> pair
                            # them into one accumulation group per bank
                            nc.tensor.matmul(
                                z_ps[:, t4, :], wzs[:, tg * TG + t4, :],
                                vblk[:, j, :],
                                start=(t4 % 2 == 0), stop=(t4 % 2 == 1))
                        ztmp = zc.tile([128, TG, D, B], F32)
                        nc.scalar.copy(
                            ztmp.rearrange("p t d b -> p (t d b)"),
                            z_ps.rearrange("p t db -> p (t db)"))
                        tmp2 = zc.tile([128, TG, D, B], F32)
                        nc.vector.tensor_mul(
                            tmp2[:], ztmp[:],
                            x_sb[:, tg * TG:(tg + 1) * TG, :, :])
                        u1 = zc.tile([128, TG, 4, B], F32)
                        nc.vector.tensor_add(
                            u1[:], tmp2[:, :, 0:4, :], tmp2[:, :, 4:8, :])
                        u2 = zc.tile([128, TG, 2, B], F32)
                        nc.vector.tensor_add(
                            u2[:], u1[:, :, 0:2, :], u1[:, :, 2:4, :])
                        bb_sl = bb[:, tg * TG:(tg + 1) * TG, j, :]
                        if it == 0:
                            nc.vector.tensor_add(
                                bb_sl, u2[:, :, 0, :], u2[:, :, 1, :])
                        else:
                            uv = zc.tile([128, TG, B], F32)
                            nc.vector.tensor_add(
                                uv[:], u2[:, :, 0, :], u2[:, :, 1, :])
                            nc.vector.tensor_add(bb_sl, bb_sl, uv[:])
    return nc


@lru_cache(maxsize=2)
def _build(n_cores):
    nc = bacc.Bacc("TRN2", target_bir_lowering=False, debug=False,
                   num_devices=n_cores)
    _emit(nc, n_cores)
    nc.compile()
    return nc


def _prep_inputs(x, W):
    """Host-side shard + relayout. Returns list of per-core input dicts."""
    x = np.asarray(x, dtype=np.float32)
    W = np.asarray(W, dtype=np.float32)
    in_maps = []
    for c in range(NCORES):
        xc = x[:, c * PL:(c + 1) * PL, :]              # (B, PL, D)
        Wc = W[:, c * PL:(c + 1) * PL, :, :]           # (J, PL, D, O)
        xr = np.ascontiguousarray(
            xc.reshape(B, T, 128, D).transpose(2, 1, 3, 0))        # [128,T,D,B]
        wsr = np.ascontiguousarray(
            Wc.reshape(J, T, 128, D, O).transpose(2, 1, 3, 0, 4))  # [128,T,D,J,O]
        wzr = np.ascontiguousarray(
            Wc.reshape(J, T, 128, D, O).transpose(0, 3, 4, 1, 2)   # j,d,o,t,p
            .reshape(J, 128, T, 128))                              # [J,(d,o),T,p]
        in_maps.append({"xt": xr, "ws": wsr, "wz": wzr})
    return in_maps


def _squash_np(s):
    sq = np.sum(s * s, axis=-1, keepdims=True)
    return s * (sq / ((1.0 + sq) * np.sqrt(sq)))


def kernel(x, W):
    nc = _build(NCORES)
    in_maps = _prep_inputs(x, W)
    res = run_bass_kernel_spmd(nc, in_maps, list(range(NCORES)))
    s3 = np.zeros((B, JO), np.float64)
    for r in res.results:
        s3 += r["s3p"].astype(np.float64)
    v = _squash_np(s3.reshape(B, J, O))
    return v.astype(np.float32)


# revision 9
# speedup vs baseline: 1.8389x; 1.8389x over previous
"""DigitCaps dynamic-routing kernel for 8 Trainium2 NeuronCores.

Problem: x(32,16384,8) f32, W(10,16384,8,16) f32 -> v(32,10,16) f32
  u_hat[b,j,p,o] = sum_d x[b,p,d] W[j,p,d,o]   (never materialized!)
  3 routing iterations (softmax over j, weighted sums over p).

Strategy: shard P=16384 over 8 cores (P_loc=2048). Per routing iteration:
  s_part[b,j,o] = sum_{p,d} (c*x)[b,j,p,d] * W[j,p,d,o]     (PE, K=p 128-chunks)
  AllReduce s (20KB) -> v = squash(s)
  z[b,j,p,d]  = sum_o W[j,p,d,o] v[b,j,o]                   (PE, K=(d,o)=128 with
                                                             block-diagonal v rhs)
  uv[b,j,p]   = sum_d x[b,p,d] z[...]                        (DVE)
  bb += uv ; c = softmax_j(bb)
Iteration 1 uses c = 0.1 exactly. Final squash + cross-core s-sum on host.
Matmul operands in bf16 (PSUM accumulation fp32); logits bb kept fp32.

Per-core SBUF layouts (p^ = p % 128 on partitions, t = p//128 in 0..15):
  xt  [128, t16, d8, b32]        ws [128, t16, d8, j10, o16]
  wz  [j10, 128=(d*16+o), t16, p128]   (all resident in SBUF, bf16)
"""
import numpy as np
import ml_dtypes
from functools import lru_cache

import concourse.bacc as bacc
import concourse.mybir as mybir
from concourse import tile
from concourse.bass_utils import run_bass_kernel_spmd

F32 = mybir.dt.float32
BF16 = mybir.dt.bfloat16
AX = mybir.AxisListType
ALU = mybir.AluOpType
ACTF = mybir.ActivationFunctionType

B, J, P, D, O = 32, 10, 16384, 8, 16
NCORES = 8
PL = P // NCORES          # 2048
T = PL // 128             # 16 tiles of 128 p's
TG = 4                    # t-group size in z-phase
JO = J * O                # 160


def _emit(nc, n_cores):
    xt = nc.dram_tensor("xt", [128, T, D, B], BF16, kind="ExternalInput")
    ws = nc.dram_tensor("ws", [128, T, D, J, O], BF16, kind="ExternalInput")
    wz = nc.dram_tensor("wz", [J, 128, T, 128], BF16, kind="ExternalInput")
    s3p = nc.dram_tensor("s3p", [B, JO], F32, kind="ExternalOutput")

    with tile.TileContext(nc) as tc:
        with (
            tc.tile_pool(name="per", bufs=1) as per,        # persistent
            tc.tile_pool(name="yp", bufs=2) as yp,
            tc.tile_pool(name="zc", bufs=2) as zc,          # z consume bufs
            tc.tile_pool(name="small", bufs=2) as small,
            tc.tile_pool(name="sps", bufs=2, space="PSUM") as sps,
            tc.tile_pool(name="zps", bufs=2, space="PSUM") as zps,
            tc.tile_pool(name="dram", bufs=2, space="DRAM") as dramp,
        ):
            x_sb = per.tile([128, T, D, B], BF16)
            nc.sync.dma_start(x_sb[:], xt[:, :, :, :])
            ws_sb = per.tile([128, T, D, J, O], BF16)
            for t in range(T):
                nc.sync.dma_start(ws_sb[:, t, :, :, :], ws[:, t, :, :, :])
            wz_sb = per.tile([128, J, T, 128], BF16)
            for j in range(J):
                nc.sync.dma_start(wz_sb[:, j, :, :], wz[j, :, :, :])
            # block-diagonal v holder: rows (d*16+o), cols per j (d*32+b).
            # Off-diagonal stays zero forever; blocks are re-DMA'd per iter.
            vblk = per.tile([128, J, D * B], BF16)
            nc.gpsimd.memset(vblk[:], 0.0)
            bb = per.tile([128, T, J, B], F32)      # routing logits
            e_sb = per.tile([128, T, J, B], BF16)   # exp(bb)
            c_sb = per.tile([128, T, J, B], BF16)   # softmax coeffs
            se = per.tile([128, T, B], F32)         # sum_j exp
            rec = per.tile([128, T, B], F32)        # 1/sum
            rec_bf = per.tile([128, T, B], BF16)

            # warmup collective: absorbs ncfw's ~30us first-collective
            # barrier while the PE is busy with the iter-0 s-phase
            wu_in = dramp.tile([B, 16], F32)
            wu_out = dramp.tile([B, 16], F32)
            wu_sb = small.tile([B, 16], F32)
            nc.vector.memset(wu_sb[:], 0.0)
            nc.sync.dma_start(wu_in[:], wu_sb[:])
            nc.gpsimd.collective_compute(
                "AllReduce", ALU.add,
                replica_groups=[list(range(n_cores))],
                ins=[wu_in[:].opt()], outs=[wu_out[:].opt()],
            )

            for it in range(3):
                # ---------------- s-phase ----------------
                s_ps = sps.tile([B, JO], F32)
                if it > 0:
                    # softmax over j: c = exp(bb) / sum_j exp(bb)
                    nc.scalar.activation(e_sb[:], bb[:], ACTF.Exp)
                    nc.vector.tensor_reduce(
                        se[:, :, :, None],
                        e_sb.rearrange("p t j b -> p t b j"),
                        AX.X, ALU.add,
                    )
                    nc.vector.reciprocal(rec[:], se[:])
                    nc.vector.tensor_copy(rec_bf[:], rec[:])
                    nc.vector.tensor_mul(
                        c_sb[:], e_sb[:],
                        rec_bf[:, :, None, :].broadcast_to([128, T, J, B]),
                    )
                for t in range(T):
                    if it == 0:
                        # c == 0.1 exactly: lhsT = x, scale folded into copy
                        for d in range(D):
                            nc.tensor.matmul(
                                s_ps[:, :],
                                x_sb[:, t, d, :],
                                ws_sb[:, t, d, :, :].rearrange("p j o -> p (j o)"),
                                start=(t == 0 and d == 0),
                                stop=(t == T - 1 and d == D - 1),
                            )
                    else:
                        y_t = yp.tile([128, J, D, B], BF16)
                        nc.vector.tensor_mul(
                            y_t[:],
                            c_sb[:, t, :, None, :].broadcast_to([128, J, D, B]),
                            x_sb[:, t, None, :, :].broadcast_to([128, J, D, B]),
                        )
                        for j in range(J):
                            for d in range(D):
                                # one accumulation group for the whole bank:
                                # start marks the 2KB zero-region pending-zero,
                                # each slice's first write then overwrites
                                nc.tensor.matmul(
                                    s_ps[:, j * O:(j + 1) * O],
                                    y_t[:, j, d, :],
                                    ws_sb[:, t, d, j, :],
                                    start=(t == 0 and j == 0 and d == 0),
                                    stop=(t == T - 1 and j == J - 1 and d == D - 1),
                                )
                s_sb = small.tile([B, JO], F32)
                nc.scalar.activation(s_sb[:], s_ps[:], ACTF.Copy,
                                     scale=0.1 if it == 0 else 1.0)
                if it == 2:
                    nc.sync.dma_start(s3p[:, :], s_sb[:])
                    break

                # ---------------- AllReduce s ----------------
                cc_in = dramp.tile([B, JO], F32)
                cc_out = dramp.tile([B, JO], F32)
                nc.sync.dma_start(cc_in[:], s_sb[:])
                nc.gpsimd.collective_compute(
                    "AllReduce", ALU.add,
                    replica_groups=[list(range(n_cores))],
                    ins=[cc_in[:].opt()], outs=[cc_out[:].opt()],
                )
                s_f = small.tile([B, JO], F32)
                nc.sync.dma_start(s_f[:], cc_out[:])

                # ---------------- squash -> v ----------------
                t2 = small.tile([B, JO], F32)
                nc.vector.tensor_mul(t2[:], s_f[:], s_f[:])
                sq = small.tile([B, J], F32)
                nc.vector.tensor_reduce(
                    sq[:, :, None], t2.rearrange("b (j o) -> b j o", j=J),
                    AX.X, ALU.add)
                r_ = small.tile([B, J], F32)
                nc.scalar.activation(r_[:], sq[:], ACTF.Sqrt)
                den = small.tile([B, J], F32)
                # den = (sq + 1) * r
                nc.vector.scalar_tensor_tensor(
                    den[:], sq[:], 1.0, r_[:], ALU.add, ALU.mult)
                rc2 = small.tile([B, J], F32)
                nc.vector.reciprocal(rc2[:], den[:])
                f_ = small.tile([B, J], F32)
                nc.vector.tensor_mul(f_[:], sq[:], rc2[:])
                v_bf = small.tile([B, J, O], BF16)
                nc.vector.tensor_mul(
                    v_bf[:], s_f.rearrange("b (j o) -> b j o", j=J),
                    f_[:, :, None].broadcast_to([B, J, O]))
                # bounce v through DRAM, then scatter transposed copies into
                # the block-diagonal slots (DMA is exempt from the 32-aligned
                # partition-start rule that engine ops have)
                v_dr = dramp.tile([B, J, O], BF16)
                nc.sync.dma_start(v_dr[:], v_bf[:])
                for j in range(J):
                    for d in range(D):
                        nc.sync.dma_start(
                            vblk[d * O:(d + 1) * O, j, d * B:(d + 1) * B],
                            v_dr[:, j, :].rearrange("b o -> o b"))

                # ---------------- z / uv phase ----------------
                for j in range(J):
                    for tg in range(T // TG):
                        z_ps = zps.tile([128, TG, D * B], F32)
                        for t4 in range(TG):
                            # two 1KB outputs share each 2KB psum bank -> pair
                            # them into one accumulation group per bank
                            nc.tensor.matmul(
                                z_ps[:, t4, :], wz_sb[:, j, tg * TG + t4, :],
                                vblk[:, j, :],
                                start=(t4 % 2 == 0), stop=(t4 % 2 == 1))
                        ztmp = zc.tile([128, TG, D, B], BF16)
                        nc.scalar.copy(
                            ztmp.rearrange("p t d b -> p (t d b)"),
                            z_ps.rearrange("p t db -> p (t db)"))
                        tmp2 = zc.tile([128, TG, D, B], BF16)
                        nc.vector.tensor_mul(
                            tmp2[:], ztmp[:],
                            x_sb[:, tg * TG:(tg + 1) * TG, :, :])
                        u1 = zc.tile([128, TG, 4, B], BF16)
                        nc.vector.tensor_add(
                            u1[:], tmp2[:, :, 0:4, :], tmp2[:, :, 4:8, :])
                        u2 = zc.tile([128, TG, 2, B], BF16)
                        nc.vector.tensor_add(
                            u2[:], u1[:, :, 0:2, :], u1[:, :, 2:4, :])
                        bb_sl = bb[:, tg * TG:(tg + 1) * TG, j, :]
                        if it == 0:
                            nc.vector.tensor_add(
                                bb_sl, u2[:, :, 0, :], u2[:, :, 1, :])
                        else:
                            uv = zc.tile([128, TG, B], F32)
                            nc.vector.tensor_add(
                                uv[:], u2[:, :, 0, :], u2[:, :, 1, :])
                            nc.vector.tensor_add(bb_sl, bb_sl, uv[:])
    return nc


@lru_cache(maxsize=2)
def _build(n_cores):
    nc = bacc.Bacc("TRN2", target_bir_lowering=False, debug=False,
                   num_devices=n_cores)
    _emit(nc, n_cores)
    nc.compile()
    return nc


def _prep_inputs(x, W):
    """Host-side shard + relayout. Returns list of per-core input dicts."""
    x = np.asarray(x, dtype=np.float32)
    W = np.asarray(W, dtype=np.float32)
    in_maps = []
    for c in range(NCORES):
        xc = x[:, c * PL:(c + 1) * PL, :]              # (B, PL, D)
        Wc = W[:, c * PL:(c + 1) * PL, :, :]           # (J, PL, D, O)
        xr = np.ascontiguousarray(
            xc.reshape(B, T, 128, D).transpose(2, 1, 3, 0)         # [128,T,D,B]
        ).astype(ml_dtypes.bfloat16)
        wsr = np.ascontiguousarray(
            Wc.reshape(J, T, 128, D, O).transpose(2, 1, 3, 0, 4)   # [128,T,D,J,O]
        ).astype(ml_dtypes.bfloat16)
        wzr = np.ascontiguousarray(
            Wc.reshape(J, T, 128, D, O).transpose(0, 3, 4, 1, 2)   # j,d,o,t,p
            .reshape(J, 128, T, 128)).astype(ml_dtypes.bfloat16)   # [J,(d,o),T,p]
        in_maps.append({"xt": xr, "ws": wsr, "wz": wzr})
    return in_maps


def _squash_np(s):
    sq = np.sum(s * s, axis=-1, keepdims=True)
    return s * (sq / ((1.0 + sq) * np.sqrt(sq)))


def kernel(x, W):
    nc = _build(NCORES)
    in_maps = _prep_inputs(x, W)
    res = run_bass_kernel_spmd(nc, in_maps, list(range(NCORES)))
    s3 = np.zeros((B, JO), np.float64)
    for r in res.results:
        s3 += r["s3p"].astype(np.float64)
    v = _squash_np(s3.reshape(B, J, O))
    return v.astype(np.float32)
